# revision 81
# baseline (speedup 1.0000x reference)
"""MLA (multi-head latent attention) forward on 8 Trainium2 NeuronCores.

Sharding: 4 cores per batch (groups [0-3], [4-7]). Core l of group g:
  - compresses kv+krope for its 512-token slice -> two in-group AllGathers
    (kv latents, then krope rows; Local outputs), overlapped with the
    replicated q compress for all 2048 batch tokens (SBUF-resident)
  - decompresses q/k/v for its 4 heads, interleaved with attention so the
    first head starts as soon as chunk-0 operands exist
  - per-head 8-core AllToAll rounds redistribute attention outputs to
    token-owner cores; outputs are pre-gated per group so each receiver
    just sums the two group-parity slots
  - projects its 512 tokens over all 16 heads, one round per AllToAll
"""

import numpy as np

import concourse.bacc as bacc
import concourse.mybir as mybir
import concourse.tile as tile
from concourse import bass_utils

B, S, D = 2, 2048, 2048
H = 16
NOPE, ROPE, VH = 128, 64, 128
HALF = ROPE // 2
QR = KVR = 512
EPS = 1e-6
W = 8            # cores
G = 4            # cores per batch group
HPC = H // G     # heads per core = 4
TC = S // G      # tokens per core slice = 512
SCALE = 1.0 / np.sqrt(NOPE + ROPE)
SQ = 512         # q supertile width
KT = 128         # k tile height
NK = S // KT     # 16 key tiles

FP16 = mybir.dt.float16
FP32 = mybir.dt.float32

_cache = {}


def _build():
    nc = bacc.Bacc("TRN2", target_bir_lowering=False, debug=False)

    def din(name, shape, dt=FP16):
        return nc.dram_tensor(name, shape, dt, kind="ExternalInput").ap()

    xT = din("xT", [D, S])                      # this batch, feature-major
    xTc = din("xTc", [D, TC])                   # this core's token slice
    w_comp = din("w_comp", [D, 1152])           # [cq 512 | ckv 512 | krope 64 | pad]
    w_dec = din("w_dec", [QR, 2048])            # [dqn 512 | dqr 256+pad | dkn 512 | dv 512]
    w_projT = din("w_projT", [H * VH, D])       # head-permuted for a2a round order
    cs4 = din("cs4", [128, S])                  # [cos; sin; -sin; cos] stacked
    cs4c = din("cs4c", [128, TC])               # this core's position slice
    dupm = din("dupm", [128, 256])              # [DUP | DUP2] maps, both part-halves
    addm = din("addm", [128, 128])              # fused row-sum + duplicate map
    tril = din("tril", [128, 128])              # causal mask block, fp16 0/1
    gates = din("gates", [128, 2], FP32)        # [gA | gB]: 1/0 by group parity
    out_c = nc.dram_tensor("out_c", [TC, D], FP32, kind="ExternalOutput").ap()

    ag_groups = [[0, 1, 2, 3], [4, 5, 6, 7]]
    a2a_groups = [[0, 1, 2, 3, 4, 5, 6, 7]]

    _scopes = []

    def _scope(name):
        if _scopes:
            _scopes[-1].__exit__(None, None, None)
            _scopes.pop()
        _scopes.append(nc.named_scope(name))
        _scopes[-1].__enter__()

    with tile.TileContext(nc) as tc:
        dram_cm = tc.tile_pool(name="dram", bufs=1, space="DRAM")
        dram = dram_cm.__enter__()
        lat_kin = dram.tile([KVR, TC], FP16, tag="lat_kin", name="lat_kin")
        lat_gkv = dram.tile([G, KVR, TC], FP16, tag="lat_gkv", name="lat_gkv")
        a2a_in = [dram.tile([W, VH, SQ], FP16, tag=f"a2a_in{r}", name=f"a2a_in{r}")
                  for r in range(HPC)]
        a2a_out = [dram.tile([W, VH, SQ], FP16, tag=f"a2a_out{r}", name=f"a2a_out{r}")
                   for r in range(HPC)]

        const_cm = tc.tile_pool(name="const", bufs=1)
        const = const_cm.__enter__()
        ones1 = const.tile([1, 128], FP16, tag="ones1", name="ones1")
        nc.any.memset(ones1[:], 1.0)
        ones_c = const.tile([128, 1], FP16, tag="ones_c", name="ones_c")
        nc.any.memset(ones_c[:], 1.0)
        invn = const.tile([128, 1], FP16, tag="invn", name="invn")
        nc.any.memset(invn[:], 1.0 / QR)
        eps_t = const.tile([1, 1], FP32, tag="eps_t", name="eps_t")
        nc.any.memset(eps_t[:], EPS)
        tril_t = const.tile([128, 128], FP16, tag="tril_t", name="tril_t")
        nc.sync.dma_start(tril_t[:], tril[:])
        cs4_t = const.tile([128, S], FP16, tag="cs4_t", name="cs4_t")
        nc.sync.dma_start(cs4_t[:], cs4[:])
        cs4c_t = const.tile([128, TC], FP16, tag="cs4c_t", name="cs4c_t")
        nc.sync.dma_start(cs4c_t[:], cs4c[:])
        dupm_t = const.tile([128, 256], FP16, tag="dupm_t", name="dupm_t")
        nc.sync.dma_start(dupm_t[:], dupm[:])
        addm_t = const.tile([128, 128], FP16, tag="addm_t", name="addm_t")
        nc.sync.dma_start(addm_t[:], addm[:])
        gates_t = const.tile([128, 2], FP32, tag="gates_t", name="gates_t")
        nc.sync.dma_start(gates_t[:], gates[:])

        def rmsnorm_store(pool, pspool, psm, key, dst_fn):
            # psm: 4 psum tiles [128, TC] holding raw latents (512 rows)
            cq_all = pool.tile([128, 4 * TC], FP32, tag="cq_a", name="cq_a", bufs=2)
            sq_all = pool.tile([128, 4 * TC], FP16, tag="sq_a", name="sq_a", bufs=2)
            for m in range(4):
                cs = slice(m * TC, (m + 1) * TC)
                nc.scalar.copy(cq_all[:, cs], psm[m][:])
                nc.vector.tensor_mul(sq_all[:, cs], cq_all[:, cs], cq_all[:, cs])
            ps_ssq = pspool.tile([1, TC], FP32, tag="ssq", name="ssq")
            for m in range(4):
                nc.tensor.matmul(ps_ssq[:], invn[:], sq_all[:, m * TC:(m + 1) * TC],
                                 start=(m == 0), stop=(m == 3))
            std_f = pool.tile([1, TC], FP32, tag=f"std_{key}", name=f"std_{key}")
            nc.scalar.activation(std_f[:], ps_ssq[:], mybir.ActivationFunctionType.Sqrt,
                                 bias=eps_t[:])
            rstd_f = pool.tile([1, TC], FP32, tag=f"rsf_{key}", name=f"rsf_{key}")
            nc.vector.reciprocal(rstd_f[:], std_f[:])
            rstd = pool.tile([1, TC], FP16, tag=f"rs_{key}", name=f"rs_{key}")
            nc.vector.tensor_copy(rstd[:], rstd_f[:])
            ps_rb = pspool.tile([128, TC], FP32, tag="rb", name="rb")
            nc.tensor.matmul(ps_rb[:], ones1[:], rstd[:], start=True, stop=True)
            for m in range(4):
                dst_fn(m, cq_all[:, m * TC:(m + 1) * TC], ps_rb)

        # ---------------- Phase 1: compress ----------------------------------
        nq_cm = tc.tile_pool(name="nq", bufs=1)
        nq_pool = nq_cm.__enter__()
        nq = [[nq_pool.tile([128, TC], FP16, tag=f"nq{k}_{cc}", name=f"nq{k}_{cc}")
               for cc in range(G)] for k in range(4)]
        krg2 = nq_pool.tile([128, S], FP16, tag="krg2", name="krg2")

        with tc.tile_pool(name="cmp_x", bufs=1) as cmp_x, \
             tc.tile_pool(name="cmp_t", bufs=1) as cmp_t, \
             tc.tile_pool(name="ps_cmp", bufs=1, space="PSUM") as ps_cmp, \
             tc.tile_pool(name="ps_nrm", bufs=1, space="PSUM") as ps_nrm:
            _scope("ph_kvcmp")
            xt = [cmp_x.tile([128, S], FP16, tag=f"xt{k}", name=f"xt{k}")
                  for k in range(16)]
            xs = [cmp_x.tile([128, TC], FP16, tag=f"xs{k}", name=f"xs{k}")
                  for k in range(16)]
            wkv = [cmp_x.tile([128, 640], FP16, tag=f"wkv{k}", name=f"wkv{k}")
                   for k in range(16)]
            wq = [cmp_x.tile([128, 512], FP16, tag=f"wq{k}", name=f"wq{k}")
                  for k in range(16)]
            for k in range(16):
                nc.scalar.dma_start(xs[k][:], xTc[k * 128:(k + 1) * 128, :])
                nc.sync.dma_start(wkv[k][:], w_comp[k * 128:(k + 1) * 128, 512:1152])

            # kv + krope for this core's slice first (feeds the AllGathers)
            psm_kv = [ps_cmp.tile([128, TC], FP32, tag=f"cm{m}", name=f"cm{m}",
                                  bufs=1) for m in range(4)]
            for k in range(16):
                for m in range(4):
                    nc.tensor.matmul(psm_kv[m][:],
                                     wkv[k][:, m * 128:(m + 1) * 128],
                                     xs[k][:], start=(k == 0), stop=(k == 15))
            for k in range(16):
                nc.sync.dma_start(wq[k][:], w_comp[k * 128:(k + 1) * 128, 0:512])
                nc.sync.dma_start(xt[k][:], xT[k * 128:(k + 1) * 128, :])

            lat_all = cmp_t.tile([128, 4 * TC], FP16, tag="lat_all", name="lat_all")

            def kv_store(m, cq_ap, ps_rb):
                nc.vector.tensor_mul(lat_all[:, m * TC:(m + 1) * TC], cq_ap, ps_rb[:])

            rmsnorm_store(cmp_t, ps_nrm, psm_kv, "kv", kv_store)
            # one transposing DMA: SBUF [p, m, t] -> DRAM [m, p, t]
            nc.scalar.dma_start(
                lat_kin[:].rearrange("(m p) t -> p m t", m=4),
                lat_all[:].rearrange("p (m t) -> p m t", m=4))

            # in-group AllGather of the kv latents (krope is computed
            # locally below from the already-resident full-batch x)
            nc.gpsimd.collective_compute(
                "AllGather",
                mybir.AluOpType.bypass,
                ins=[lat_kin[:].rearrange("a b -> (a b)")],
                outs=[lat_gkv[:].rearrange("w a b -> (w a b)")],
                replica_groups=ag_groups,
            )

            _scope("ph_qcmp")
            # q compress, replicated over all 4 chunks of the batch
            for cc in range(G):
                psm_q = [ps_cmp.tile([128, TC], FP32, tag=f"cm{m}", name=f"cm{m}",
                                     bufs=1) for m in range(4)]
                ccs = slice(cc * TC, (cc + 1) * TC)
                for k in range(16):
                    for m in range(4):
                        nc.tensor.matmul(psm_q[m][:], wq[k][:, m * 128:(m + 1) * 128],
                                         xt[k][:, ccs], start=(k == 0), stop=(k == 15))

                def q_store(m, cq_ap, ps_rb, cc=cc):
                    nc.vector.tensor_mul(nq[m][cc][:], cq_ap, ps_rb[:])

                rmsnorm_store(cmt := cmp_t, ps_nrm, psm_q, f"q{cc}", q_store)

            # local krope for all batch tokens: compress -> dup -> *cs4 ->
            # fused row-sum+duplicate, filling krg2 with no collective involved
            for cc in range(G):
                ccs = slice(cc * TC, (cc + 1) * TC)
                ps_kr = ps_cmp.tile([ROPE, TC], FP32, tag="ckr", name="ckr", bufs=1)
                for k in range(16):
                    nc.tensor.matmul(ps_kr[:], wkv[k][:, 512:576], xt[k][:, ccs],
                                     start=(k == 0), stop=(k == 15))
                kr_f = cmp_t.tile([ROPE, TC], FP16, tag="kr_f", name="kr_f", bufs=2)
                nc.scalar.copy(kr_f[:], ps_kr[:])
                ps_kd = ps_nrm.tile([128, TC], FP32, tag="rb", name="kd")
                nc.tensor.matmul(ps_kd[:], dupm_t[0:ROPE, 0:128], kr_f[:],
                                 start=True, stop=True)
                kru = cmp_t.tile([128, TC], FP16, tag="kru", name="kru", bufs=2)
                nc.vector.tensor_mul(kru[:], ps_kd[:], cs4_t[:, ccs])
                ps_krr = ps_nrm.tile([128, TC], FP32, tag="ssq", name="krr")
                nc.tensor.matmul(ps_krr[:], addm_t[:], kru[:], start=True, stop=True)
                nc.scalar.copy(krg2[:, ccs], ps_krr[:])

        # persistent attention operands (filled by interleaved decompress)
        attn_cm = tc.tile_pool(name="attn", bufs=1)
        attn_pool = attn_cm.__enter__()
        qn = [attn_pool.tile([NOPE, S], FP16, tag=f"qn{h}", name=f"qn{h}")
              for h in range(HPC)]
        qr_ = [attn_pool.tile([128, S], FP16, tag=f"qr{h}", name=f"qr{h}")
               for h in range(HPC)]
        kn = [attn_pool.tile([NOPE, S], FP16, tag=f"kn{h}", name=f"kn{h}")
              for h in range(HPC)]
        val = attn_pool.tile([128, NK * 512], FP16, tag="val", name="val")
        # val: token-tile j occupies cols j*512:(j+1)*512 (4 heads x 128 feats)

        dec_cm = tc.tile_pool(name="dec_w", bufs=1)
        dec_w = dec_cm.__enter__()
        wd = [dec_w.tile([128, 2048], FP16, tag=f"wd{k}", name=f"wd{k}")
              for k in range(4)]
        for k in range(4):
            nc.sync.dma_start(wd[k][:], w_dec[k * 128:(k + 1) * 128, :])
        nkv = [[dec_w.tile([128, TC], FP16, tag=f"nkv{k}_{cc}", name=f"nkv{k}_{cc}")
                for cc in range(G)] for k in range(4)]
        ps_dec_cm = tc.tile_pool(name="ps_dec", bufs=4, space="PSUM")
        ps_dec = ps_dec_cm.__enter__()
        dect_cm = tc.tile_pool(name="dec_t", bufs=1)
        dec_t = dect_cm.__enter__()
        for cc in range(G):
            for k in range(4):
                nc.scalar.dma_start(nkv[k][cc][:],
                                    lat_gkv[cc, k * 128:(k + 1) * 128, :])

        _scope("ph_dec")

        def dec_qn(h, ccs, pool=None, tag="lin"):
            for cc in ccs:
                ps = (pool or ps_dec).tile([128, TC], FP32, tag=tag, name="lin")
                for k in range(4):
                    nc.tensor.matmul(ps[:], wd[k][:, h * NOPE:(h + 1) * NOPE],
                                     nq[k][cc][:], start=(k == 0), stop=(k == 3))
                nc.vector.tensor_copy(qn[h][:, cc * TC:(cc + 1) * TC], ps[:])

        def dec_qr(p, ccs, pool=None, tag="lin"):
            # pair p covers heads 2p, 2p+1
            for cc in ccs:
                ps = (pool or ps_dec).tile([128, TC], FP32, tag=tag, name="lin")
                for k in range(4):
                    nc.tensor.matmul(ps[:], wd[k][:, 512 + p * 128:512 + (p + 1) * 128],
                                     nq[k][cc][:], start=(k == 0), stop=(k == 3))
                qr_pre = (dec_t if pool is None else _att_state["att_t"]).tile(
                    [128, TC], FP16, tag="qr_pre", name="qr_pre", bufs=2)
                nc.vector.tensor_copy(qr_pre[:], ps[:])
                cs = slice(cc * TC, (cc + 1) * TC)
                for hh in range(2):
                    ps_qd = (pool or ps_dec).tile([128, TC], FP32, tag=tag, name="lin")
                    nc.tensor.matmul(ps_qd[:],
                                     dupm_t[hh * ROPE:(hh + 1) * ROPE, 0:128],
                                     qr_pre[hh * ROPE:(hh + 1) * ROPE, :],
                                     start=True, stop=True)
                    nc.vector.tensor_mul(qr_[p * 2 + hh][:, cs], ps_qd[:],
                                         cs4_t[:, cs])

        def dec_kn(h, ccs, pool=None, tag="lin"):
            for cc in ccs:
                ps = (pool or ps_dec).tile([128, TC], FP32, tag=tag, name="lin")
                for k in range(4):
                    nc.tensor.matmul(ps[:],
                                     wd[k][:, 1024 + h * NOPE:1024 + (h + 1) * NOPE],
                                     nkv[k][cc][:], start=(k == 0), stop=(k == 3))
                if pool is None:
                    nc.vector.tensor_copy(kn[h][:, cc * TC:(cc + 1) * TC], ps[:])
                else:
                    # mid-attention: DVE carries z-adds; use the scalar engine
                    nc.scalar.copy(kn[h][:, cc * TC:(cc + 1) * TC], ps[:])

        def dec_val(js):
            for j in js:
                cc, jj = j // 4, j % 4
                ps = ps_dec.tile([128, 512], FP32, tag="lin", name="lin")
                for k in range(4):
                    nc.tensor.matmul(ps[:], nkv[k][cc][:, jj * 128:(jj + 1) * 128],
                                     wd[k][:, 1536:2048], start=(k == 0), stop=(k == 3))
                nc.vector.tensor_copy(val[:, j * 512:(j + 1) * 512], ps[:])

        # ---------------- attention + a2a + proj (interleaved) ---------------
        _att_state = {}

        def att_pools_open():
            _att_state["att_cm"] = tc.tile_pool(name="att_t", bufs=1)
            _att_state["att_t"] = _att_state["att_cm"].__enter__()
            _att_state["ps_s2_cm"] = tc.tile_pool(name="ps_s2", bufs=2, space="PSUM")
            _att_state["ps_s2"] = _att_state["ps_s2_cm"].__enter__()
            _att_state["ps_s1_cm"] = tc.tile_pool(name="ps_s1", bufs=2, space="PSUM")
            _att_state["ps_s1"] = _att_state["ps_s1_cm"].__enter__()
            _att_state["ps_av_cm"] = tc.tile_pool(name="ps_av", bufs=2, space="PSUM")
            _att_state["ps_av"] = _att_state["ps_av_cm"].__enter__()

        wp_rounds = []

        def finalize_z(acc):
            # z-sum, reciprocal, and its broadcast; independent of pav
            att_t = _att_state["att_t"]
            ps_s1 = _att_state["ps_s1"]
            pz = ps_s1.tile([1, SQ], FP32, tag="s1", name="z")
            nc.tensor.matmul(pz[:], ones_c[:], acc[:], start=True, stop=True)
            rz_f = att_t.tile([1, SQ], FP32, tag="rzf", name="rzf", bufs=2)
            nc.vector.reciprocal(rz_f[:], pz[:])
            rz = att_t.tile([1, SQ], FP16, tag="rz", name="rz", bufs=2)
            nc.vector.tensor_copy(rz[:], rz_f[:])
            prb = ps_s1.tile([128, SQ], FP32, tag="s1", name="rbb")
            nc.tensor.matmul(prb[:], ones1[:], rz[:], start=True, stop=True)
            prb_s = att_t.tile([128, SQ], FP16, tag="prbs", name="prbs", bufs=2)
            nc.scalar.copy(prb_s[:], prb[:])
            return prb_s

        def finalize(fin):
            # normalize + gate + ship one supertile's attention output
            h, Q, pav, prb = fin
            att_t = _att_state["att_t"]
            ao = att_t.tile([128, SQ], FP16, tag="ao", name="ao", bufs=2)
            nc.vector.tensor_mul(ao[:], pav[:], prb[:])
            aoA = att_t.tile([128, SQ], FP16, tag="aoA", name="aoA", bufs=2)
            aoB = att_t.tile([128, SQ], FP16, tag="aoB", name="aoB", bufs=2)
            nc.scalar.activation(aoA[:], ao[:], mybir.ActivationFunctionType.Copy,
                                 scale=gates_t[:, 0:1])
            nc.vector.tensor_scalar_mul(aoB[:], ao[:], gates_t[:, 1:2])
            nc.sync.dma_start(a2a_in[h][Q, :, :], aoA[:])
            nc.sync.dma_start(a2a_in[h][G + Q, :, :], aoB[:])

        pending_fin = [None]

        def attention_head(h):
            att_t = _att_state["att_t"]
            ps_s2 = _att_state["ps_s2"]
            ps_s1 = _att_state["ps_s1"]
            ps_av = _att_state["ps_av"]
            for Q in range(G):
                nkt = 4 * Q + 4
                pav = ps_av.tile([128, SQ], FP32, tag="av", name="av")
                acc = att_t.tile([128, SQ], FP16, tag="zacc", name="zacc", bufs=3)
                first_pav = [True]
                pend = []  # deferred pav matmuls: (pT, ap_col0, col0, val-slice)

                def flush_pav(upto):
                    while len(pend) > upto:
                        pT, a0, c0, vs = pend.pop(0)
                        nc.tensor.matmul(pav[:, c0:SQ], val[:, vs], pT[:, a0:a0 + SQ - c0],
                                         start=first_pav[0], stop=False)
                        first_pav[0] = False

                # non-diagonal key tiles, two per psum tile / exp instruction
                for pi in range(2 * Q):
                    ps2 = ps_s2.tile([128, 2 * SQ], FP32, tag="s2", name="s2")
                    pT2 = att_t.tile([128, 2 * SQ], FP16, tag="pT2", name="pT2", bufs=6)
                    qs = slice(Q * SQ, (Q + 1) * SQ)
                    for half in range(2):
                        kt = 2 * pi + half
                        ks = slice(kt * KT, (kt + 1) * KT)
                        hs = slice(half * SQ, (half + 1) * SQ)
                        nc.tensor.matmul(ps2[:, hs], kn[h][:, ks], qn[h][:, qs],
                                         start=True, stop=False)
                        nc.tensor.matmul(ps2[:, hs], krg2[:, ks], qr_[h][:, qs],
                                         start=False, stop=True)
                    flush_pav(4)
                    if pi == 1 and pending_fin[0] is not None:
                        finalize(pending_fin[0])
                        pending_fin[0] = None
                    nc.scalar.activation(pT2[:], ps2[:],
                                         mybir.ActivationFunctionType.Exp)
                    for half in range(2):
                        kt = 2 * pi + half
                        hs = slice(half * SQ, (half + 1) * SQ)
                        if kt == 0:
                            nc.vector.tensor_copy(acc[:], pT2[:, hs])
                        else:
                            nc.vector.tensor_add(acc[:], acc[:], pT2[:, hs])
                        pend.append((pT2, half * SQ, 0,
                                     slice(kt * 512 + h * 128,
                                           kt * 512 + (h + 1) * 128)))
                # diagonal key tiles, column-sliced
                for diag in range(4):
                    kt = 4 * Q + diag
                    c0 = diag * KT
                    qs = slice(Q * SQ + c0, (Q + 1) * SQ)
                    pss = ps_s1.tile([128, SQ], FP32, tag="s1", name="s1")
                    ks = slice(kt * KT, (kt + 1) * KT)
                    nc.tensor.matmul(pss[:, c0:SQ], kn[h][:, ks], qn[h][:, qs],
                                     start=True, stop=False)
                    nc.tensor.matmul(pss[:, c0:SQ], krg2[:, ks], qr_[h][:, qs],
                                     start=False, stop=True)
                    flush_pav(3)
                    if Q == 0 and diag == 2 and pending_fin[0] is not None:
                        finalize(pending_fin[0])
                        pending_fin[0] = None
                    pT = att_t.tile([128, SQ], FP16, tag="pT", name="pT", bufs=4)
                    nc.scalar.activation(pT[:, c0:SQ], pss[:, c0:SQ],
                                         mybir.ActivationFunctionType.Exp)
                    nc.vector.tensor_mul(pT[:, c0:c0 + KT], pT[:, c0:c0 + KT],
                                         tril_t[:])
                    if kt == 0:
                        nc.vector.tensor_copy(acc[:], pT[:])
                    else:
                        nc.vector.tensor_add(acc[:, c0:SQ], acc[:, c0:SQ],
                                             pT[:, c0:SQ])
                    pend.append((pT, c0, c0,
                                 slice(kt * 512 + h * 128, kt * 512 + (h + 1) * 128)))
                prb = finalize_z(acc)
                flush_pav(1)
                # last pav closes the accumulation group
                pT, a0, c0, vs = pend.pop(0)
                nc.tensor.matmul(pav[:, c0:SQ], val[:, vs], pT[:, a0:a0 + SQ - c0],
                                 start=first_pav[0], stop=True)
                if pending_fin[0] is not None:
                    finalize(pending_fin[0])
                pending_fin[0] = (h, Q, pav, prb)
            # flush before the collective so its input DMAs exist
            finalize(pending_fin[0])
            pending_fin[0] = None
            nc.gpsimd.collective_compute(
                "AllToAll",
                mybir.AluOpType.bypass,
                ins=[a2a_in[h][:].rearrange("w a b -> (w a b)")],
                outs=[a2a_out[h][:].rearrange("w a b -> (w a b)")],
                replica_groups=a2a_groups,
            )
            # stream this round's proj weights under attention
            wp_r = [prj.tile([128, D], FP16, tag=f"wpk{s}", name=f"wpk{s}", bufs=1)
                    for s in range(4)]
            for s in range(4):
                k = 4 * h + s
                nc.sync.dma_start(wp_r[s][:], w_projT[k * 128:(k + 1) * 128, :])
            wp_rounds.append(wp_r)


        def proj_unit(r, ao_r, ms_local, mt, nt):
            if r == 3 and (mt * 4 + nt) % 2 == 1:
                # attention is over; the idle s2 banks double the rotation depth
                ps = _att_state["ps_s2"].tile([128, 2 * SQ], FP32, tag="s2",
                                              name="pj2")[:, 0:512]
            else:
                ps = _att_state["ps_s1"].tile([128, 512], FP32, tag="s1", name="pj")
            for s in range(G):
                nc.tensor.matmul(ps[:], ao_r[s][:, ms_local],
                                 wp_rounds[r][s][:, nt * 512:(nt + 1) * 512],
                                 start=(s == 0), stop=(s == 3))
            if r == 0:
                nc.vector.tensor_copy(acc_p[mt][nt][:], ps[:])
            elif r < 3:
                nc.vector.tensor_add(acc_p[mt][nt][:], acc_p[mt][nt][:], ps[:])
            else:
                ms = slice(mt * 128, (mt + 1) * 128)
                ev = ev_t.tile([128, 512], FP32, tag="ev", name="ev")
                nc.vector.tensor_add(ev[:], acc_p[mt][nt][:], ps[:])
                nc.sync.dma_start(out_c[ms, nt * 512:(nt + 1) * 512], ev[:])

        def proj_round(r):
            # fetch this round's a2a output; sum the two group-parity slots
            ao_r = [prj.tile([128, TC], FP16, tag=f"aot{s}", name=f"aot{s}", bufs=1)
                    for s in range(G)]
            for s in range(G):
                if r < 3:
                    nc.gpsimd.dma_start(ao_r[s][:], a2a_out[r][s, :, :])
                    nc.gpsimd.dma_start(ao_r[s][:], a2a_out[r][G + s, :, :],
                                        accum_op=mybir.AluOpType.add)
                else:
                    # HWDGE queues are empty at the tail and issue ~2x faster
                    aux = prj.tile([128, TC], FP16, tag="aox", name="aox", bufs=2)
                    nc.sync.dma_start(ao_r[s][:], a2a_out[r][s, :, :])
                    nc.scalar.dma_start(aux[:], a2a_out[r][G + s, :, :])
                    nc.vector.tensor_add(ao_r[s][:], ao_r[s][:], aux[:])
            for mt in range(4):
                for nt in range(4):
                    proj_unit(r, ao_r, slice(mt * 128, (mt + 1) * 128), mt, nt)

        # interleaved decompress + attention emission: head 0's operands first
        dec_qn(0, range(G))
        dec_qr(0, range(G))
        dec_kn(0, range(G))
        dec_val(range(NK))
        dect_cm.__exit__(None, None, None)
        ps_dec_cm.__exit__(None, None, None)
        att_pools_open()
        prj_cm = tc.tile_pool(name="prj", bufs=1)
        prj = prj_cm.__enter__()
        acc_p = [[prj.tile([128, 512], FP16, tag=f"accp{mt}_{nt}", name=f"accp{mt}_{nt}")
                  for nt in range(4)] for mt in range(4)]
        ev_cm = tc.tile_pool(name="ev_t", bufs=6)
        ev_t = ev_cm.__enter__()
        _scope("ph_att")
        attention_head(0)
        _scope("ph_dec2")
        dec_qn(1, range(G), pool=_att_state["ps_s1"], tag="s1")
        dec_kn(1, range(G), pool=_att_state["ps_s1"], tag="s1")
        _scope("ph_att2")
        attention_head(1)
        _scope("ph_dec3")
        dec_qr(1, range(G), pool=_att_state["ps_s1"], tag="s1")
        dec_qn(2, range(G), pool=_att_state["ps_s1"], tag="s1")
        dec_kn(2, range(G), pool=_att_state["ps_s1"], tag="s1")
        _scope("ph_att3")
        attention_head(2)
        _scope("ph_dec4")
        dec_qn(3, range(G), pool=_att_state["ps_s1"], tag="s1")
        dec_kn(3, range(G), pool=_att_state["ps_s1"], tag="s1")
        _scope("ph_att4")
        proj_round(0)
        attention_head(3)
        proj_round(1)
        proj_round(2)
        proj_round(3)

        if _scopes:
            _scopes[-1].__exit__(None, None, None)
            _scopes.pop()

        ev_cm.__exit__(None, None, None)
        prj_cm.__exit__(None, None, None)
        _att_state["ps_av_cm"].__exit__(None, None, None)
        _att_state["ps_s1_cm"].__exit__(None, None, None)
        _att_state["ps_s2_cm"].__exit__(None, None, None)
        _att_state["att_cm"].__exit__(None, None, None)
        dec_cm.__exit__(None, None, None)
        attn_cm.__exit__(None, None, None)
        nq_cm.__exit__(None, None, None)
        const_cm.__exit__(None, None, None)
        dram_cm.__exit__(None, None, None)

    nc.compile()
    return nc


def _prep_inputs(x, freqs_cis, w_cq, w_qnorm, w_dqn, w_dqr, w_ckv, w_kvnorm, w_dkn,
                 w_dv, w_krope, w_proj):
    perm = np.concatenate([np.arange(0, ROPE, 2), np.arange(1, ROPE, 2)])
    f16 = np.float16

    xT_b = [np.ascontiguousarray(x[g].T.astype(f16)) for g in range(B)]  # (D, S)

    w_comp = np.zeros((D, 1152), f16)
    w_comp[:, 0:512] = w_cq.T.astype(f16)
    w_comp[:, 512:1024] = w_ckv.T.astype(f16)
    w_comp[:, 1024:1088] = ((w_krope / H)[perm, :].T).astype(f16)

    wdqn = (w_dqn * w_qnorm[None, :] * SCALE).reshape(H, NOPE, QR)
    wdqr = ((w_dqr * w_qnorm[None, :] * SCALE).reshape(H, ROPE, QR))[:, perm, :]
    wdkn = (w_dkn * w_kvnorm[None, :]).reshape(H, NOPE, KVR)
    wdv = (w_dv * w_kvnorm[None, :]).reshape(H, VH, KVR)

    # proj weight rows permuted to a2a arrival order: block k=r*4+s <-> head 4s+r
    wpp = np.empty((H * VH, D), np.float32)
    for r in range(HPC):
        for s in range(G):
            hd = 4 * s + r
            wpp[(r * 4 + s) * VH:(r * 4 + s + 1) * VH, :] = \
                w_proj[:, hd * VH:(hd + 1) * VH].T
    wpp = np.ascontiguousarray(wpp.astype(f16))

    cosT = freqs_cis[:, :, 0].T.astype(np.float32)  # (HALF, S)
    sinT = freqs_cis[:, :, 1].T.astype(np.float32)
    cs4 = np.ascontiguousarray(np.vstack([cosT, sinT, -sinT, cosT]).astype(f16))
    # DUP: [64]->[x0;x0;x1;x1], DUP2: [64]->[r;r], ADDM: [128]-> row p + row p+64
    dup = np.zeros((ROPE, 128), np.float32)
    for p in range(128):
        fp = p if p < 32 else (p - 32 if p < 96 else p - 64)
        dup[fp, p] = 1.0
    dup2 = np.zeros((ROPE, 128), np.float32)
    for p in range(128):
        dup2[p % 64, p] = 1.0
    dupm = np.hstack([dup, dup2]).astype(f16)
    dupm = np.ascontiguousarray(np.vstack([dupm, dupm]))
    addm = np.zeros((128, 128), np.float32)
    for r in range(128):
        for p in range(128):
            if r % 64 == p % 64:
                addm[r, p] = 1.0
    addm = np.ascontiguousarray(addm.astype(f16))

    # tril[k, q'] = 1 iff q' >= k (used on the diagonal 128x128 block)
    tril = (np.arange(128)[None, :] >= np.arange(128)[:, None]).astype(f16)

    in_maps = []
    for c in range(W):
        g, l = c // G, c % G
        hs = slice(l * HPC, (l + 1) * HPC)
        w_dec = np.zeros((QR, 2048), f16)
        w_dec[:, 0:512] = wdqn[hs].reshape(HPC * NOPE, QR).T.astype(f16)
        w_dec[:, 512:512 + HPC * ROPE] = wdqr[hs].reshape(HPC * ROPE, QR).T.astype(f16)
        w_dec[:, 1024:1536] = wdkn[hs].reshape(HPC * NOPE, KVR).T.astype(f16)
        w_dec[:, 1536:2048] = wdv[hs].reshape(HPC * VH, KVR).T.astype(f16)
        sc = slice(l * TC, (l + 1) * TC)
        gates = np.zeros((128, 2), np.float32)
        gates[:, 0] = 1.0 if g == 0 else 0.0
        gates[:, 1] = 1.0 if g == 1 else 0.0
        in_maps.append({
            "xT": xT_b[g],
            "xTc": np.ascontiguousarray(xT_b[g][:, sc]),
            "w_comp": w_comp,
            "w_dec": w_dec,
            "w_projT": wpp,
            "cs4": cs4,
            "cs4c": np.ascontiguousarray(cs4[:, sc]),
            "dupm": dupm,
            "addm": addm,
            "tril": tril,
            "gates": gates,
        })
    return in_maps


last_results = None


def kernel(x, mask, freqs_cis, w_cq, w_qnorm, w_dqn, w_dqr, w_ckv, w_kvnorm, w_dkn,
           w_dv, w_krope, w_proj):
    global last_results
    if "nc" not in _cache:
        _cache["nc"] = _build()
    nc = _cache["nc"]

    in_maps = _prep_inputs(np.asarray(x, np.float32), np.asarray(freqs_cis, np.float32),
                           np.asarray(w_cq, np.float32), np.asarray(w_qnorm, np.float32),
                           np.asarray(w_dqn, np.float32), np.asarray(w_dqr, np.float32),
                           np.asarray(w_ckv, np.float32), np.asarray(w_kvnorm, np.float32),
                           np.asarray(w_dkn, np.float32), np.asarray(w_dv, np.float32),
                           np.asarray(w_krope, np.float32), np.asarray(w_proj, np.float32))

    res = bass_utils.run_bass_kernel_spmd(nc, in_maps, core_ids=list(range(W)))
    last_results = res

    out = np.empty((B, S, D), np.float32)
    for c in range(W):
        g, l = c // G, c % G
        out[g, l * TC:(l + 1) * TC, :] = res.results[c]["out_c"]
    return out
